# revision 1
# baseline (speedup 1.0000x reference)
"""PixPro loss kernel for 8 Trainium2 NeuronCores.

Data-parallel over batch: 1024 samples -> 128 per core (= SBUF partitions).

Heavy part (cos similarity over 512 channels x 49 grid points):
  host transposes per-core features to [B=128, N=49, C=512] (c contiguous);
  samples stay on SBUF partitions. Per grid point n, a single fused
  scalar_tensor_tensor with accum_out computes the channel reduction in one
  pass: dot (b*m), |b|^2, |m|^2 each via one DVE pass. No PE, no PSUM.
  Feature DMAs are triggered from the idle sync (SP) queue for lookahead.

Mask part (grids / distances / masks) runs with samples on partitions and
overlaps the feature DMAs. Per-core output is [128, 2] = (masked loss sum
contribution, intersection flag); host does the final psum + divide.
"""

import sys

import numpy as np

if "/opt/trn_rl_repo" not in sys.path:
    sys.path.insert(0, "/opt/trn_rl_repo")

B = 1024
C = 512
S = 7
N = S * S  # 49
NCORES = 8
BP = B // NCORES  # 128 samples per core
NBLK = 7  # n-blocks of 7 grid points each
IO_BUFS = 4
EPS = 1e-6
THRESH2 = 0.7 * 0.7

_t = np.linspace(0.0, 1.0, S).astype(np.float32)
_n = np.arange(N)
TX_TAB = np.ascontiguousarray(np.tile(_t[_n // S], (BP, 1)))  # [128, 49]
TY_TAB = np.ascontiguousarray(np.tile(_t[_n % S], (BP, 1)))  # [128, 49]

ALL_PARTS = frozenset(("mask", "heavy", "tail"))

_NC = None


def _emit(tc, d, parts=ALL_PARTS):
    """Emit the tile kernel. d: dict of DRAM APs."""
    from contextlib import ExitStack

    from concourse import mybir

    nc = tc.nc
    f32 = mybir.dt.float32
    A = mybir.AluOpType
    AX = mybir.AxisListType

    with ExitStack() as ctx:
        pers = ctx.enter_context(tc.tile_pool(name="pers", bufs=1))
        io = ctx.enter_context(tc.tile_pool(name="io", bufs=IO_BUFS))
        work = ctx.enter_context(tc.tile_pool(name="work", bufs=1))

        # ---- persistent small tiles ----
        pb_t = pers.tile([BP, 4], f32, tag="pb_t")
        pm_t = pers.tile([BP, 4], f32, tag="pm_t")
        fb_t = pers.tile([BP, 1], f32, tag="fb_t")
        fm_t = pers.tile([BP, 1], f32, tag="fm_t")
        tx_t = pers.tile([BP, N], f32, tag="tx_t")
        ty_t = pers.tile([BP, N], f32, tag="ty_t")

        nc.gpsimd.dma_start(pb_t[:], d["pb"][:])
        nc.gpsimd.dma_start(pm_t[:], d["pm"][:])
        nc.gpsimd.dma_start(fb_t[:], d["fb"][:])
        nc.gpsimd.dma_start(fm_t[:], d["fm"][:])
        nc.gpsimd.dma_start(tx_t[:], d["tx"][:])
        nc.gpsimd.dma_start(ty_t[:], d["ty"][:])

        xb = pb_t[:, 0:1]
        yb = pb_t[:, 1:2]
        wb = pb_t[:, 2:3]
        hb = pb_t[:, 3:4]
        xm = pm_t[:, 0:1]
        ym = pm_t[:, 1:2]
        wm = pm_t[:, 2:3]
        hm = pm_t[:, 3:4]

        out_sb = pers.tile([BP, 2], f32, tag="out_sb")

        if "mask" in parts:
            # ---- mask part (samples on partitions) ----
            # flip: y' = y + h*f, h' = h*(1 - 2f)
            yb2 = pers.tile([BP, 1], f32, tag="yb2")
            hb2 = pers.tile([BP, 1], f32, tag="hb2")
            ym2 = pers.tile([BP, 1], f32, tag="ym2")
            hm2 = pers.tile([BP, 1], f32, tag="hm2")
            tmp1 = pers.tile([BP, 1], f32, tag="tmp1")
            nc.vector.scalar_tensor_tensor(yb2[:], fb_t[:], hb, yb, A.mult, A.add)
            nc.vector.tensor_scalar(tmp1[:], fb_t[:], -2.0, 1.0, A.mult, A.add)
            nc.vector.tensor_tensor(hb2[:], tmp1[:], hb, A.mult)
            nc.vector.scalar_tensor_tensor(ym2[:], fm_t[:], hm, ym, A.mult, A.add)
            nc.vector.tensor_scalar(tmp1[:], fm_t[:], -2.0, 1.0, A.mult, A.add)
            nc.vector.tensor_tensor(hm2[:], tmp1[:], hm, A.mult)

            # grids [BP, N]
            gxb = pers.tile([BP, N], f32, tag="gxb")
            gyb = pers.tile([BP, N], f32, tag="gyb")
            gxm = pers.tile([BP, N], f32, tag="gxm")
            gym = pers.tile([BP, N], f32, tag="gym")
            nc.vector.tensor_scalar(gxb[:], tx_t[:], wb, xb, A.mult, A.add)
            nc.vector.tensor_scalar(
                gyb[:], ty_t[:], hb2[:, 0:1], yb2[:, 0:1], A.mult, A.add
            )
            nc.vector.tensor_scalar(gxm[:], tx_t[:], wm, xm, A.mult, A.add)
            nc.vector.tensor_scalar(
                gym[:], ty_t[:], hm2[:, 0:1], ym2[:, 0:1], A.mult, A.add
            )

            # tau^2 = 0.49 * (w^2 + h^2) per side
            tau2b = pers.tile([BP, 1], f32, tag="tau2b")
            tau2m = pers.tile([BP, 1], f32, tag="tau2m")
            nc.vector.tensor_tensor(tmp1[:], wb, wb, A.mult)
            nc.vector.scalar_tensor_tensor(tau2b[:], hb, hb, tmp1[:], A.mult, A.add)
            nc.vector.tensor_scalar_mul(tau2b[:], tau2b[:], THRESH2)
            nc.vector.tensor_tensor(tmp1[:], wm, wm, A.mult)
            nc.vector.scalar_tensor_tensor(tau2m[:], hm, hm, tmp1[:], A.mult, A.add)
            nc.vector.tensor_scalar_mul(tau2m[:], tau2m[:], THRESH2)

            # D2[p, i, j] = (gxb_i - gxm_j)^2 + (gyb_i - gym_j)^2
            t0 = pers.tile([BP, N, N], f32, tag="t0")
            t1 = pers.tile([BP, N, N], f32, tag="t1")
            t2 = pers.tile([BP, N, N], f32, tag="t2")
            gxb_i = gxb[:].unsqueeze(2).broadcast_to([BP, N, N])
            gxm_j = gxm[:].unsqueeze(1).broadcast_to([BP, N, N])
            gyb_i = gyb[:].unsqueeze(2).broadcast_to([BP, N, N])
            gym_j = gym[:].unsqueeze(1).broadcast_to([BP, N, N])
            nc.vector.tensor_tensor(t0[:], gxb_i, gxm_j, A.subtract)  # dx
            nc.vector.tensor_tensor(t1[:], gyb_i, gym_j, A.subtract)  # dy
            nc.vector.tensor_tensor(t2[:], t0[:], t0[:], A.mult)  # dx^2
            nc.scalar.square(t0[:], t1[:])  # dy^2 (ACT)
            nc.vector.tensor_tensor(t1[:], t2[:], t0[:], A.add)  # D2 -> t1

            # masks + counts + mask marginals
            nnzb = pers.tile([BP, 1], f32, tag="nnzb")
            nnzm = pers.tile([BP, 1], f32, tag="nnzm")
            colsum_b = pers.tile([BP, N], f32, tag="colsum_b")  # sum_i mask_b[i, j]
            rowsum_m = pers.tile([BP, N], f32, tag="rowsum_m")  # sum_j mask_m[i, j]
            nc.vector.tensor_scalar(
                t2[:], t1[:], tau2b[:, 0:1], None, A.is_lt, op1=A.add,
                accum_out=nnzb[:],
            )
            nc.vector.tensor_reduce(
                colsum_b[:], t2[:].transpose([0, 2, 1]), AX.X, A.add
            )
            nc.vector.tensor_scalar(
                t2[:], t1[:], tau2m[:, 0:1], None, A.is_lt, op1=A.add,
                accum_out=nnzm[:],
            )
            nc.vector.tensor_reduce(rowsum_m[:], t2[:], AX.X, A.add)

            # intersection flag: (2|cx1-cx2| < wb+wm) & (2|cy1-cy2| < hb+hm)
            # (uses raw p_base/p_moment, not flipped)
            u1 = pers.tile([BP, 1], f32, tag="u1")
            u2 = pers.tile([BP, 1], f32, tag="u2")
            okx = pers.tile([BP, 1], f32, tag="okx")
            oky = pers.tile([BP, 1], f32, tag="oky")
            inter = pers.tile([BP, 1], f32, tag="inter")
            nc.vector.scalar_tensor_tensor(u1[:], wb, 0.5, xb, A.mult, A.add)
            nc.vector.scalar_tensor_tensor(u2[:], wm, 0.5, xm, A.mult, A.add)
            nc.vector.tensor_tensor(u1[:], u1[:], u2[:], A.subtract)
            nc.scalar.activation(u1[:], u1[:], mybir.ActivationFunctionType.Abs)
            nc.vector.tensor_tensor(u2[:], wb, wm, A.add)
            nc.vector.scalar_tensor_tensor(okx[:], u1[:], 2.0, u2[:], A.mult, A.is_lt)
            nc.vector.scalar_tensor_tensor(u1[:], hb, 0.5, yb, A.mult, A.add)
            nc.vector.scalar_tensor_tensor(u2[:], hm, 0.5, ym, A.mult, A.add)
            nc.vector.tensor_tensor(u1[:], u1[:], u2[:], A.subtract)
            nc.scalar.activation(u1[:], u1[:], mybir.ActivationFunctionType.Abs)
            nc.vector.tensor_tensor(u2[:], hb, hm, A.add)
            nc.vector.scalar_tensor_tensor(oky[:], u1[:], 2.0, u2[:], A.mult, A.is_lt)
            nc.vector.tensor_tensor(inter[:], okx[:], oky[:], A.mult)

        if "heavy" in parts:
            # ---- heavy part: fused multiply+channel-reduce per grid point ----
            dot_sb = pers.tile([BP, N], f32, tag="dot_sb")
            nrm_b = pers.tile([BP, N], f32, tag="nrm_b")
            nrm_m = pers.tile([BP, N], f32, tag="nrm_m")
            scr_d = work.tile([BP, C], f32, tag="scr_d")
            scr_a = work.tile([BP, C], f32, tag="scr_a")
            nblk = N // NBLK
            for blk in range(NBLK):
                n0 = blk * nblk
                b_t = io.tile([BP, nblk, C], f32, tag="b_t")
                m_t = io.tile([BP, nblk, C], f32, tag="m_t")
                h = nblk // 2
                nc.sync.dma_start(b_t[:, :h, :], d["bt"][:, n0 : n0 + h, :])
                nc.sync.dma_start(m_t[:, :h, :], d["mt"][:, n0 : n0 + h, :])
                nc.sync.dma_start(b_t[:, h:, :], d["bt"][:, n0 + h : n0 + nblk, :])
                nc.sync.dma_start(m_t[:, h:, :], d["mt"][:, n0 + h : n0 + nblk, :])
                for j in range(nblk):
                    n = n0 + j
                    # dot on DVE (fused multiply + channel accum)
                    nc.vector.scalar_tensor_tensor(
                        scr_d[:], b_t[:, j, :], 1.0, m_t[:, j, :],
                        A.mult, A.mult, accum_out=dot_sb[:, n : n + 1],
                    )
                    # |b|^2 on ACT (fused square + channel accum)
                    nc.scalar.activation(
                        scr_a[:], b_t[:, j, :],
                        mybir.ActivationFunctionType.Square,
                        accum_out=nrm_b[:, n : n + 1],
                    )
                    # |m|^2: split between DVE and ACT to balance engines
                    if n % 3 == 0:
                        nc.vector.scalar_tensor_tensor(
                            scr_d[:], m_t[:, j, :], 1.0, m_t[:, j, :],
                            A.mult, A.mult, accum_out=nrm_m[:, n : n + 1],
                        )
                    else:
                        nc.scalar.activation(
                            scr_a[:], m_t[:, j, :],
                            mybir.ActivationFunctionType.Square,
                            accum_out=nrm_m[:, n : n + 1],
                        )

        do_ttr = "tail" in parts or "ttr" in parts
        do_cos = do_ttr or "cos" in parts
        if do_cos:
            # ---- cos assembly ----
            den = pers.tile([BP, N], f32, tag="den")
            cos_t = pers.tile([BP, N], f32, tag="cos_t")
            nc.scalar.sqrt(nrm_b[:], nrm_b[:])
            nc.scalar.sqrt(nrm_m[:], nrm_m[:])
            nc.vector.tensor_scalar_max(nrm_b[:], nrm_b[:], EPS)
            nc.vector.tensor_scalar_max(nrm_m[:], nrm_m[:], EPS)
            nc.vector.tensor_tensor(den[:], nrm_b[:], nrm_m[:], A.mult)
            nc.vector.reciprocal(den[:], den[:])
            nc.vector.tensor_tensor(cos_t[:], dot_sb[:], den[:], A.mult)

        if do_ttr:
            # s_b = sum_j cos[j]*colsum_b[j]; s_m = sum_i cos[i]*rowsum_m[i]
            sb_s = pers.tile([BP, 1], f32, tag="sb_s")
            sm_s = pers.tile([BP, 1], f32, tag="sm_s")
            scr = pers.tile([BP, N], f32, tag="scr")
            nc.vector.tensor_tensor(scr[:], cos_t[:], colsum_b[:], A.mult)
            nc.vector.tensor_reduce(sb_s[:], scr[:], AX.X, A.add)
            nc.vector.tensor_tensor(scr[:], cos_t[:], rowsum_m[:], A.mult)
            nc.vector.tensor_reduce(sm_s[:], scr[:], AX.X, A.add)

        if "tail" in parts:
            # loss = s / max(nnz, 1) per side; contribution = (lb+lm)*inter
            lb = pers.tile([BP, 1], f32, tag="lb")
            lm = pers.tile([BP, 1], f32, tag="lm")
            nc.vector.tensor_scalar_max(nnzb[:], nnzb[:], 1.0)
            nc.vector.tensor_scalar_max(nnzm[:], nnzm[:], 1.0)
            nc.vector.reciprocal(nnzb[:], nnzb[:])
            nc.vector.reciprocal(nnzm[:], nnzm[:])
            nc.vector.tensor_tensor(lb[:], sb_s[:], nnzb[:], A.mult)
            nc.vector.tensor_tensor(lm[:], sm_s[:], nnzm[:], A.mult)
            nc.vector.tensor_tensor(lb[:], lb[:], lm[:], A.add)
            nc.vector.tensor_tensor(lb[:], lb[:], inter[:], A.mult)

            nc.vector.tensor_copy(out_sb[:, 0:1], lb[:])
            nc.vector.tensor_copy(out_sb[:, 1:2], inter[:])
        elif do_ttr:
            nc.vector.tensor_copy(out_sb[:, 0:1], sb_s[:])
            nc.vector.tensor_copy(out_sb[:, 1:2], sm_s[:])
        elif do_cos:
            nc.vector.tensor_copy(out_sb[:, 0:1], cos_t[:, 0:1])
            nc.vector.tensor_copy(out_sb[:, 1:2], den[:, 0:1])
        elif "mask" in parts:
            nc.vector.tensor_copy(out_sb[:, 0:1], nnzb[:])
            nc.vector.tensor_copy(out_sb[:, 1:2], inter[:])
        elif "heavy" in parts:
            nc.vector.tensor_copy(out_sb[:, 0:1], dot_sb[:, 0:1])
            nc.vector.tensor_copy(out_sb[:, 1:2], nrm_b[:, 0:1])
        else:
            nc.vector.tensor_copy(out_sb[:, 0:2], pb_t[:, 0:2])

        nc.gpsimd.dma_start(d["o"][:], out_sb[:])


def build(debug=False, parts=ALL_PARTS):
    import concourse.bacc as bacc
    import concourse.tile as tile
    from concourse import mybir

    nc = bacc.Bacc(
        "TRN2",
        target_bir_lowering=False,
        debug=debug,
        enable_asserts=False,
        num_devices=NCORES,
    )
    f32 = mybir.dt.float32
    d = {
        "bt": nc.dram_tensor("bt", [BP, N, C], f32, kind="ExternalInput").ap(),
        "mt": nc.dram_tensor("mt", [BP, N, C], f32, kind="ExternalInput").ap(),
        "pb": nc.dram_tensor("pb", [BP, 4], f32, kind="ExternalInput").ap(),
        "pm": nc.dram_tensor("pm", [BP, 4], f32, kind="ExternalInput").ap(),
        "fb": nc.dram_tensor("fb", [BP, 1], f32, kind="ExternalInput").ap(),
        "fm": nc.dram_tensor("fm", [BP, 1], f32, kind="ExternalInput").ap(),
        "tx": nc.dram_tensor("tx", [BP, N], f32, kind="ExternalInput").ap(),
        "ty": nc.dram_tensor("ty", [BP, N], f32, kind="ExternalInput").ap(),
        "o": nc.dram_tensor("o", [BP, 2], f32, kind="ExternalOutput").ap(),
    }
    with tile.TileContext(nc) as tc:
        _emit(tc, d, parts)
    nc.compile()
    return nc


def make_in_maps(base, moment, p_base, p_moment, f_base, f_moment):
    in_maps = []
    for k in range(NCORES):
        sl = slice(k * BP, (k + 1) * BP)
        bt = np.ascontiguousarray(
            np.asarray(base[sl], dtype=np.float32).reshape(BP, C, N).transpose(0, 2, 1)
        )
        mt = np.ascontiguousarray(
            np.asarray(moment[sl], dtype=np.float32)
            .reshape(BP, C, N)
            .transpose(0, 2, 1)
        )
        in_maps.append(
            {
                "bt": bt,
                "mt": mt,
                "pb": np.ascontiguousarray(np.asarray(p_base[sl], dtype=np.float32)),
                "pm": np.ascontiguousarray(np.asarray(p_moment[sl], dtype=np.float32)),
                "fb": np.ascontiguousarray(np.asarray(f_base[sl], dtype=np.float32)),
                "fm": np.ascontiguousarray(np.asarray(f_moment[sl], dtype=np.float32)),
                "tx": TX_TAB,
                "ty": TY_TAB,
            }
        )
    return in_maps


def reduce_outputs(per_core_outs):
    """per_core_outs: list of [128, 2] arrays -> final scalar loss."""
    allo = np.concatenate([np.asarray(o, dtype=np.float64) for o in per_core_outs])
    pos = allo[:, 0].sum()
    cnt = allo[:, 1].sum()
    return np.asarray(-pos / max(cnt, 1.0), dtype=np.float32)


def kernel(base, moment, p_base, p_moment, f_base, f_moment, _trace=False):
    global _NC
    from concourse.bass_utils import run_bass_kernel_spmd

    if _NC is None:
        _NC = build()
    in_maps = make_in_maps(base, moment, p_base, p_moment, f_base, f_moment)
    res = run_bass_kernel_spmd(_NC, in_maps, core_ids=list(range(NCORES)), trace=_trace)
    out = reduce_outputs([r["o"] for r in res.results])
    if _trace:
        return out, res
    return out



# revision 2
# speedup vs baseline: 1.0133x; 1.0133x over previous
"""PixPro loss kernel v3 for 8 Trainium2 NeuronCores.

Data-parallel over batch: 1024 samples -> 128 per core (= SBUF partitions).
Features stream as fp16 [128, 49, 512]; per 12/13-point DMA chunk:
  - DVE: products b*m (fp16 TT, 2x) then fold chain 512->32 (fp16 adds, 2x)
    and a grouped tensor_reduce straight into the per-point sums.
  - ACT: squares of b/m for most points (elementwise, folded by DVE the
    same way); REBAL points per chunk per tensor instead use one
    ACT Square+accum_out per point (keeps DVE free) to balance engines.
Mask part is separable (D2[i,j] = Dx2 + Dy2 outer add) and runs during the
feature DMAs; is_lt masks + transposed reduces give colsum/rowsum and nnz
early, so the final s_b/s_m are tiny [128,49] dots after cos.
Output [128, 2] = (loss contribution, intersection); host sums + divides.
"""

import sys

import numpy as np

if "/opt/trn_rl_repo" not in sys.path:
    sys.path.insert(0, "/opt/trn_rl_repo")

B = 1024
C = 512
S = 7
N = S * S  # 49
NCORES = 8
BP = B // NCORES  # 128
EPS = 1e-6
THRESH2 = 0.7 * 0.7

# (offset, pts, rebal): rebal pts per tensor go to ACT Square+accum per point
CHUNKS = ((0, 6, 4), (6, 7, 5), (13, 12, 7), (25, 12, 7), (37, 8, 3), (45, 4, 0))
MAXPT = 12
REBAL = 5  # max rebal (for scratch sizing)

_T7 = np.linspace(0.0, 1.0, S).astype(np.float32)
TR_TAB = np.ascontiguousarray(np.tile(_T7, (BP, 1)))  # [128, 7]

_NC = None


def _emit(tc, d):
    from contextlib import ExitStack

    from concourse import mybir

    nc = tc.nc
    f32 = mybir.dt.float32
    f16 = mybir.dt.float16
    A = mybir.AluOpType
    AX = mybir.AxisListType
    ACTF = mybir.ActivationFunctionType

    with ExitStack() as ctx:
        pers = ctx.enter_context(tc.tile_pool(name="pers", bufs=1))

        bt = pers.tile([BP, N, C], f16, tag="bt")
        mt = pers.tile([BP, N, C], f16, tag="mt")
        s0 = pers.tile([BP, MAXPT, C], f16, tag="s0")
        FP = 7
        sq_b0 = pers.tile([BP, FP, C], f16, tag="sq_b0", name="sq_b0")
        sq_b1 = pers.tile([BP, FP, C], f16, tag="sq_b1", name="sq_b1")
        sq_m0 = pers.tile([BP, FP, C], f16, tag="sq_m0", name="sq_m0")
        sq_m1 = pers.tile([BP, FP, C], f16, tag="sq_m1", name="sq_m1")
        sq_b = [sq_b0, sq_b1]
        sq_m = [sq_m0, sq_m1]
        ppa = pers.tile([BP, C], f16, tag="ppa")
        ppb = pers.tile([BP, C], f16, tag="ppb")
        t1 = pers.tile([BP, MAXPT, C // 2], f16, tag="t1")
        t2 = pers.tile([BP, MAXPT, C // 4], f16, tag="t2")
        t3 = pers.tile([BP, MAXPT, C // 8], f16, tag="t3")
        t4 = pers.tile([BP, MAXPT, C // 16], f16, tag="t4")

        dot = pers.tile([BP, N], f32, tag="dot")
        ssb = pers.tile([BP, N], f32, tag="ssb")
        ssm = pers.tile([BP, N], f32, tag="ssm")

        pb_t = pers.tile([BP, 4], f32, tag="pb_t")
        pm_t = pers.tile([BP, 4], f32, tag="pm_t")
        fb_t = pers.tile([BP, 1], f32, tag="fb_t")
        fm_t = pers.tile([BP, 1], f32, tag="fm_t")
        tr_t = pers.tile([BP, S], f32, tag="tr_t")

        nc.sync.dma_start(pb_t[:], d["pb"][:])
        nc.sync.dma_start(pm_t[:], d["pm"][:])
        nc.sync.dma_start(fb_t[:], d["fb"][:])
        nc.sync.dma_start(fm_t[:], d["fm"][:])
        nc.sync.dma_start(tr_t[:], d["tr"][:])

        for n0, pts, _ in CHUNKS:
            nc.sync.dma_start(bt[:, n0 : n0 + pts, :], d["bt"][:, n0 : n0 + pts, :])
            nc.sync.dma_start(mt[:, n0 : n0 + pts, :], d["mt"][:, n0 : n0 + pts, :])

        xb = pb_t[:, 0:1]
        yb = pb_t[:, 1:2]
        wb = pb_t[:, 2:3]
        hb = pb_t[:, 3:4]
        xm = pm_t[:, 0:1]
        ym = pm_t[:, 1:2]
        wm = pm_t[:, 2:3]
        hm = pm_t[:, 3:4]

        # ---- mask part (fp32 smalls; overlaps feature DMAs) ----
        yb2 = pers.tile([BP, 1], f32, tag="yb2")
        hb2 = pers.tile([BP, 1], f32, tag="hb2")
        ym2 = pers.tile([BP, 1], f32, tag="ym2")
        hm2 = pers.tile([BP, 1], f32, tag="hm2")
        tmp1 = pers.tile([BP, 1], f32, tag="tmp1")
        nc.vector.scalar_tensor_tensor(yb2[:], fb_t[:], hb, yb, A.mult, A.add)
        nc.vector.tensor_scalar(tmp1[:], fb_t[:], -2.0, 1.0, A.mult, A.add)
        nc.vector.tensor_tensor(hb2[:], tmp1[:], hb, A.mult)
        nc.vector.scalar_tensor_tensor(ym2[:], fm_t[:], hm, ym, A.mult, A.add)
        nc.vector.tensor_scalar(tmp1[:], fm_t[:], -2.0, 1.0, A.mult, A.add)
        nc.vector.tensor_tensor(hm2[:], tmp1[:], hm, A.mult)

        rxb = pers.tile([BP, S], f32, tag="rxb")
        rxm = pers.tile([BP, S], f32, tag="rxm")
        cyb = pers.tile([BP, S], f32, tag="cyb")
        cym = pers.tile([BP, S], f32, tag="cym")
        nc.vector.tensor_scalar(rxb[:], tr_t[:], wb, xb, A.mult, A.add)
        nc.vector.tensor_scalar(rxm[:], tr_t[:], wm, xm, A.mult, A.add)
        nc.vector.tensor_scalar(cyb[:], tr_t[:], hb2[:, 0:1], yb2[:, 0:1], A.mult, A.add)
        nc.vector.tensor_scalar(cym[:], tr_t[:], hm2[:, 0:1], ym2[:, 0:1], A.mult, A.add)

        dx2 = pers.tile([BP, S, S], f32, tag="dx2")
        dy2 = pers.tile([BP, S, S], f32, tag="dy2")
        nc.vector.tensor_tensor(
            dx2[:],
            rxb[:].unsqueeze(2).broadcast_to([BP, S, S]),
            rxm[:].unsqueeze(1).broadcast_to([BP, S, S]),
            A.subtract,
        )
        nc.vector.tensor_tensor(dx2[:], dx2[:], dx2[:], A.mult)
        nc.vector.tensor_tensor(
            dy2[:],
            cyb[:].unsqueeze(2).broadcast_to([BP, S, S]),
            cym[:].unsqueeze(1).broadcast_to([BP, S, S]),
            A.subtract,
        )
        nc.vector.tensor_tensor(dy2[:], dy2[:], dy2[:], A.mult)

        # D2 [BP, (i1 j1), (i2 j2)] via loop over j1 (<=3 free dims per op)
        d2 = pers.tile([BP, N, N], f32, tag="d2")
        _mask_part_queue = []
        def _defer(fn):
            _mask_part_queue.append(fn)
        def _run_deferred():
            for fn in _mask_part_queue:
                fn()
            _mask_part_queue.clear()
        d2v = d2[:].rearrange("p (i1 j1) n2 -> p i1 j1 n2", j1=S)
        dx2v = dx2[:].unsqueeze(3).broadcast_to([BP, S, S, S])
        def _emit_d2():
            for j1 in range(S):
                nc.vector.tensor_tensor(
                    d2v[:, :, j1, :],
                    dx2v,
                    dy2[:, j1, :].unsqueeze(1).unsqueeze(2).broadcast_to([BP, S, S, S]),
                    A.add,
                )
        _defer(_emit_d2)

        tau2b = pers.tile([BP, 1], f32, tag="tau2b")
        tau2m = pers.tile([BP, 1], f32, tag="tau2m")
        nc.vector.tensor_tensor(tmp1[:], wb, wb, A.mult)
        nc.vector.scalar_tensor_tensor(tau2b[:], hb, hb, tmp1[:], A.mult, A.add)
        nc.vector.tensor_scalar_mul(tau2b[:], tau2b[:], THRESH2)
        nc.vector.tensor_tensor(tmp1[:], wm, wm, A.mult)
        nc.vector.scalar_tensor_tensor(tau2m[:], hm, hm, tmp1[:], A.mult, A.add)
        nc.vector.tensor_scalar_mul(tau2m[:], tau2m[:], THRESH2)

        # masks (fp16) + nnz (accum) + colsum/rowsum via reduces
        nnzb = pers.tile([BP, 1], f32, tag="nnzb")
        nnzm = pers.tile([BP, 1], f32, tag="nnzm")
        mb = pers.tile([BP, N, N], f16, tag="mb")
        mm = pers.tile([BP, N, N], f16, tag="mm")
        colsum_b = pers.tile([BP, N], f32, tag="colsum_b")
        rowsum_m = pers.tile([BP, N], f32, tag="rowsum_m")
        cfold = pers.tile([BP, 25, N], f16, tag="cfold")
        def _emit_mask_b():
            nc.vector.tensor_scalar(
                mb[:], d2[:], tau2b[:, 0:1], None, A.is_lt, op1=A.add,
                accum_out=nnzb[:],
            )
            # colsum_b[j] = sum_i mb[i, j]: fold over i at fp16 2x
            nc.vector.tensor_tensor(
                cfold[:, :24, :], mb[:, 0:24, :], mb[:, 24:48, :], A.add
            )
            nc.vector.tensor_tensor(
                cfold[:, :12, :], cfold[:, 0:12, :], cfold[:, 12:24, :], A.add
            )
            nc.vector.tensor_tensor(
                cfold[:, :6, :], cfold[:, 0:6, :], cfold[:, 6:12, :], A.add
            )
            nc.vector.tensor_tensor(
                cfold[:, :3, :], cfold[:, 0:3, :], cfold[:, 3:6, :], A.add
            )
            nc.vector.tensor_tensor(
                cfold[:, 1, :], cfold[:, 1, :], cfold[:, 2, :], A.add
            )
            nc.vector.tensor_tensor(
                cfold[:, 0, :], cfold[:, 0, :], cfold[:, 1, :], A.add
            )
            nc.vector.scalar_tensor_tensor(
                colsum_b[:], mb[:, 48, :], 1.0, cfold[:, 0, :], A.mult, A.add
            )
        def _emit_mask_m():
            nc.vector.tensor_scalar(
                mm[:], d2[:], tau2m[:, 0:1], None, A.is_lt, op1=A.add,
                accum_out=nnzm[:],
            )
            nc.vector.tensor_reduce(rowsum_m[:], mm[:], AX.X, A.add)
        _defer(_emit_mask_b)
        _defer(_emit_mask_m)

        # intersection flag
        u1 = pers.tile([BP, 1], f32, tag="u1")
        u2 = pers.tile([BP, 1], f32, tag="u2")
        okx = pers.tile([BP, 1], f32, tag="okx")
        inter = pers.tile([BP, 1], f32, tag="inter")
        nc.vector.scalar_tensor_tensor(u1[:], wb, 0.5, xb, A.mult, A.add)
        nc.vector.scalar_tensor_tensor(u2[:], wm, 0.5, xm, A.mult, A.add)
        nc.vector.tensor_tensor(u1[:], u1[:], u2[:], A.subtract)
        nc.scalar.activation(u1[:], u1[:], ACTF.Abs)
        nc.vector.tensor_tensor(u2[:], wb, wm, A.add)
        nc.vector.scalar_tensor_tensor(okx[:], u1[:], 2.0, u2[:], A.mult, A.is_lt)
        nc.vector.scalar_tensor_tensor(u1[:], hb, 0.5, yb, A.mult, A.add)
        nc.vector.scalar_tensor_tensor(u2[:], hm, 0.5, ym, A.mult, A.add)
        nc.vector.tensor_tensor(u1[:], u1[:], u2[:], A.subtract)
        nc.scalar.activation(u1[:], u1[:], ACTF.Abs)
        nc.vector.tensor_tensor(u2[:], hb, hm, A.add)
        nc.vector.scalar_tensor_tensor(inter[:], u1[:], 2.0, u2[:], A.mult, A.is_lt)
        nc.vector.tensor_tensor(inter[:], okx[:], inter[:], A.mult)

        # ---- heavy part ----
        def chain(src, dst, n0, pts):
            """Fold src [BP, pts, 512] -> reduce into dst[:, n0:n0+pts]."""
            h1, h2, h3, h4 = C // 2, C // 4, C // 8, C // 16
            nc.vector.tensor_tensor(
                t1[:, :pts, :], src[:, :pts, :h1], src[:, :pts, h1:], A.add
            )
            nc.vector.tensor_tensor(
                t2[:, :pts, :], t1[:, :pts, :h2], t1[:, :pts, h2:], A.add
            )
            nc.vector.tensor_tensor(
                t3[:, :pts, :], t2[:, :pts, :h3], t2[:, :pts, h3:], A.add
            )
            nc.vector.tensor_tensor(
                t4[:, :pts, :], t3[:, :pts, :h4], t3[:, :pts, h4:], A.add
            )
            nc.vector.tensor_reduce(
                dst[:, n0 : n0 + pts], t4[:, :pts, :], AX.X, A.add
            )

        for c, (n0, pts, rebal) in enumerate(CHUNKS):
            fold_pts = pts - rebal
            bt_c = bt[:, n0 : n0 + pts, :]
            mt_c = mt[:, n0 : n0 + pts, :]
            sb_t = sq_b[c % 2]
            sm_t = sq_m[c % 2]
            # per-point ACT Square + accum (dedicated scratch; no DVE dep)
            for j in range(fold_pts, pts):
                n = n0 + j
                nc.scalar.activation(
                    ppa[:], bt[:, n, :], ACTF.Square, accum_out=ssb[:, n : n + 1]
                )
                nc.scalar.activation(
                    ppb[:], mt[:, n, :], ACTF.Square, accum_out=ssm[:, n : n + 1]
                )
            # giant squares into ping-pong scratch
            nc.scalar.activation(
                sb_t[:, :fold_pts, :], bt[:, n0 : n0 + fold_pts, :], ACTF.Square
            )
            nc.scalar.activation(
                sm_t[:, :fold_pts, :], mt[:, n0 : n0 + fold_pts, :], ACTF.Square
            )
            # DVE: dot products + chains
            nc.vector.tensor_tensor(s0[:, :pts, :], bt_c, mt_c, A.mult)
            chain(s0, dot, n0, pts)
            chain(sb_t, ssb, n0, fold_pts)
            chain(sm_t, ssm, n0, fold_pts)
            if c == 0 and _mask_part_queue:
                _mask_part_queue.pop(0)()  # D2 build
            elif c == 2:
                _run_deferred()  # masks + colsum/rowsum
        _run_deferred()

        # cos = dot / sqrt(ssb * ssm)
        den = pers.tile([BP, N], f32, tag="den")
        cos_t = pers.tile([BP, N], f32, tag="cos_t")
        nc.vector.tensor_tensor(den[:], ssb[:], ssm[:], A.mult)
        nc.vector.tensor_scalar_max(den[:], den[:], EPS * EPS * EPS * EPS)
        nc.scalar.activation(den[:], den[:], ACTF.Sqrt)
        nc.vector.reciprocal(den[:], den[:])
        nc.vector.tensor_tensor(cos_t[:], dot[:], den[:], A.mult)

        # s_b = <cos, colsum_b>; s_m = <cos, rowsum_m>
        sb_s = pers.tile([BP, 1], f32, tag="sb_s")
        sm_s = pers.tile([BP, 1], f32, tag="sm_s")
        scr = pers.tile([BP, N], f32, tag="scr")
        nc.vector.tensor_tensor(scr[:], cos_t[:], colsum_b[:], A.mult)
        nc.vector.tensor_reduce(sb_s[:], scr[:], AX.X, A.add)
        nc.vector.tensor_tensor(scr[:], cos_t[:], rowsum_m[:], A.mult)
        nc.vector.tensor_reduce(sm_s[:], scr[:], AX.X, A.add)

        # tail
        out_sb = pers.tile([BP, 2], f32, tag="out_sb")
        lb = pers.tile([BP, 1], f32, tag="lb")
        lm = pers.tile([BP, 1], f32, tag="lm")
        nc.vector.tensor_scalar_max(nnzb[:], nnzb[:], 1.0)
        nc.vector.tensor_scalar_max(nnzm[:], nnzm[:], 1.0)
        nc.vector.reciprocal(nnzb[:], nnzb[:])
        nc.vector.reciprocal(nnzm[:], nnzm[:])
        nc.vector.tensor_tensor(lb[:], sb_s[:], nnzb[:], A.mult)
        nc.vector.tensor_tensor(lm[:], sm_s[:], nnzm[:], A.mult)
        nc.vector.tensor_tensor(lb[:], lb[:], lm[:], A.add)
        nc.vector.tensor_tensor(lb[:], lb[:], inter[:], A.mult)
        nc.vector.tensor_copy(out_sb[:, 0:1], lb[:])
        nc.vector.tensor_copy(out_sb[:, 1:2], inter[:])

        nc.sync.dma_start(d["o"][:], out_sb[:])


def build(debug=False):
    import concourse.bacc as bacc
    import concourse.tile as tile
    from concourse import mybir

    nc = bacc.Bacc(
        "TRN2",
        target_bir_lowering=False,
        debug=debug,
        enable_asserts=False,
        num_devices=NCORES,
    )
    f32 = mybir.dt.float32
    f16 = mybir.dt.float16
    d = {
        "bt": nc.dram_tensor("bt", [BP, N, C], f16, kind="ExternalInput").ap(),
        "mt": nc.dram_tensor("mt", [BP, N, C], f16, kind="ExternalInput").ap(),
        "pb": nc.dram_tensor("pb", [BP, 4], f32, kind="ExternalInput").ap(),
        "pm": nc.dram_tensor("pm", [BP, 4], f32, kind="ExternalInput").ap(),
        "fb": nc.dram_tensor("fb", [BP, 1], f32, kind="ExternalInput").ap(),
        "fm": nc.dram_tensor("fm", [BP, 1], f32, kind="ExternalInput").ap(),
        "tr": nc.dram_tensor("tr", [BP, S], f32, kind="ExternalInput").ap(),
        "o": nc.dram_tensor("o", [BP, 2], f32, kind="ExternalOutput").ap(),
    }
    with tile.TileContext(nc) as tc:
        _emit(tc, d)
    nc.compile()
    return nc


def make_in_maps(base, moment, p_base, p_moment, f_base, f_moment):
    in_maps = []
    for k in range(NCORES):
        sl = slice(k * BP, (k + 1) * BP)
        bt = (
            np.asarray(base[sl], dtype=np.float32)
            .reshape(BP, C, N)
            .transpose(0, 2, 1)
            .astype(np.float16)
        )
        mt = (
            np.asarray(moment[sl], dtype=np.float32)
            .reshape(BP, C, N)
            .transpose(0, 2, 1)
            .astype(np.float16)
        )
        in_maps.append(
            {
                "bt": np.ascontiguousarray(bt),
                "mt": np.ascontiguousarray(mt),
                "pb": np.ascontiguousarray(np.asarray(p_base[sl], dtype=np.float32)),
                "pm": np.ascontiguousarray(np.asarray(p_moment[sl], dtype=np.float32)),
                "fb": np.ascontiguousarray(np.asarray(f_base[sl], dtype=np.float32)),
                "fm": np.ascontiguousarray(np.asarray(f_moment[sl], dtype=np.float32)),
                "tr": TR_TAB,
            }
        )
    return in_maps


def reduce_outputs(per_core_outs):
    allo = np.concatenate([np.asarray(o, dtype=np.float64) for o in per_core_outs])
    pos = allo[:, 0].sum()
    cnt = allo[:, 1].sum()
    return np.asarray(-pos / max(cnt, 1.0), dtype=np.float32)


def kernel(base, moment, p_base, p_moment, f_base, f_moment, _trace=False):
    global _NC
    from concourse.bass_utils import run_bass_kernel_spmd

    if _NC is None:
        _NC = build()
    in_maps = make_in_maps(base, moment, p_base, p_moment, f_base, f_moment)
    res = run_bass_kernel_spmd(_NC, in_maps, core_ids=list(range(NCORES)), trace=_trace)
    out = reduce_outputs([r["o"] for r in res.results])
    if _trace:
        return out, res
    return out
